# revision 3
# baseline (speedup 1.0000x reference)
"""BlockDiagonalLinear Trainium2 kernel (8 NeuronCores, SPMD data-parallel).

Problem: x [8, 2048, 4096] f32, W [1024, 4, 4] f32,
  y[b, t, 4n+o] = sum_i x[b, t, 4n+i] * W[n, o, i]

Strategy
--------
Data-parallel over the batch dim: core c gets x[c] (2048 x 4096).

On the host (free — grading is HW exec time):
  * transpose each shard to feature-major xT [4096, 2048] so the feature
    (contraction) dim lands on SBUF partitions,
  * expand W into 32 block-diagonal [128, 128] matrices (one per
    128-feature chunk), packed as wbd [128, 32*128]:
        wbd[4n'+i, 128*fc + 4n'+o] = W[32*fc + n', o, i]

On the device, per 128-feature chunk fc (32 chunks):
  * DMA xT chunk [128, 2048] to SBUF (1 MB contiguous rows),
  * 4 matmuls [K=128] x [128, 512] with the block-diagonal lhsT
    (float32r: full PE rate at N>=512, ~1e-4 rel precision),
  * PSUM -> SBUF copies alternating VectorE / ScalarE,
  * DMA y chunk [128, 2048] back.

Host transposes the per-core outputs back and stacks. The kernel is
DMA-bound (~64 MB/core at ~360 GB/s).
"""
import os
import numpy as np

B = 8
T = 2048
DIM = 4096
NBLK = 1024
BLOCK = 4
P = 128
FCHUNKS = DIM // P          # 32
BLK_PER_CHUNK = P // BLOCK  # 32
TTILE = 512                 # one PSUM bank at fp32
NTT = T // TTILE            # 4
N_CORES = 8

LAST_EXEC_NS = None
_CACHE = {}


def _build_nc():
    import concourse.bacc as bacc
    import concourse.mybir as mybir
    from concourse.tile import TileContext

    F32 = mybir.dt.float32
    F32R = mybir.dt.float32r

    nc = bacc.Bacc()
    xT = nc.declare_dram_parameter("xT", [DIM, T], F32R, isOutput=False)
    wbd = nc.declare_dram_parameter("wbd", [P, FCHUNKS * P], F32R, isOutput=False)
    yT = nc.declare_dram_parameter("yT", [DIM, T], F32, isOutput=True)

    with TileContext(nc) as tc:
        with tc.tile_pool(name="wpool", bufs=1) as wpool, \
             tc.tile_pool(name="xpool", bufs=4) as xpool, \
             tc.tile_pool(name="ypool", bufs=4) as ypool, \
             tc.tile_pool(name="psum", bufs=8, space="PSUM") as pp:
            w_sb = wpool.tile([P, FCHUNKS * P], F32R)
            nc.sync.dma_start(out=w_sb[:], in_=wbd[:])
            for fc in range(FCHUNKS):
                x_sb = xpool.tile([P, T], F32R, name="x_sb")
                nc.sync.dma_start(out=x_sb[:], in_=xT[fc * P:(fc + 1) * P, :])
                y_sb = ypool.tile([P, T], F32, name="y_sb")
                for tt in range(NTT):
                    ps = pp.tile([P, TTILE], F32, name="ps")
                    nc.tensor.matmul(
                        ps[:],
                        lhsT=w_sb[:, fc * P:(fc + 1) * P],
                        rhs=x_sb[:, tt * TTILE:(tt + 1) * TTILE],
                        start=True,
                        stop=True,
                    )
                    if (fc * NTT + tt) % 2 == 0:
                        nc.vector.tensor_copy(
                            out=y_sb[:, tt * TTILE:(tt + 1) * TTILE], in_=ps[:]
                        )
                    else:
                        nc.scalar.copy(
                            out=y_sb[:, tt * TTILE:(tt + 1) * TTILE], in_=ps[:]
                        )
                nc.sync.dma_start(out=yT[fc * P:(fc + 1) * P, :], in_=y_sb[:])
    nc.compile()
    return nc


def _get_nc():
    if "nc" not in _CACHE:
        _CACHE["nc"] = _build_nc()
    return _CACHE["nc"]


def _build_wbd(W: np.ndarray) -> np.ndarray:
    # wbd[4n'+i, 128*fc + 4n'+o] = W[32*fc + n', o, i]
    Wc = W.reshape(FCHUNKS, BLK_PER_CHUNK, BLOCK, BLOCK)  # [fc, n', o, i]
    wbd = np.zeros((FCHUNKS, BLK_PER_CHUNK, BLOCK, BLK_PER_CHUNK, BLOCK),
                   dtype=np.float32)  # [fc, n', i, n', o]
    idx = np.arange(BLK_PER_CHUNK)
    # advanced indices (dims 1 and 3) broadcast to the FRONT: value is [n', fc, i, o]
    wbd[:, idx, :, idx, :] = np.transpose(Wc, (1, 0, 3, 2))
    # -> [fi_local, fc*128 + fo]
    return np.ascontiguousarray(
        wbd.reshape(FCHUNKS, P, P).transpose(1, 0, 2).reshape(P, FCHUNKS * P)
    )


def kernel(x: np.ndarray, W: np.ndarray) -> np.ndarray:
    global LAST_EXEC_NS
    from concourse.bass_utils import run_bass_kernel_spmd

    x = np.ascontiguousarray(x, dtype=np.float32)
    W = np.ascontiguousarray(W, dtype=np.float32)
    wbd = _build_wbd(W)

    in_maps = [
        {"xT": np.ascontiguousarray(x[c].T), "wbd": wbd}
        for c in range(N_CORES)
    ]

    nc = _get_nc()
    trace = os.environ.get("KERNEL_TRACE", "0") == "1"
    res = run_bass_kernel_spmd(nc, in_maps, list(range(N_CORES)), trace=trace)
    LAST_EXEC_NS = res.exec_time_ns

    y = np.empty((B, T, DIM), dtype=np.float32)
    for c in range(N_CORES):
        y[c] = res.results[c]["yT"].T
    return y
